# revision 1
# baseline (speedup 1.0000x reference)
"""Trainium2 Bass kernel for a post-LN transformer encoder block.

Problem: x[2,2048,1024], 16 heads, FFN 4096, mask all-False.

Sharding (zero-collective sequence parallel):
  8 cores = 2 batch elements x 4 query slices of 512 tokens.
  Each core computes K/V for the full 2048-token sequence of its batch
  element (replicated within the 4-core group), attention + FFN for its
  512 query tokens only. Host pre-transposes/casts inputs and stitches
  the 8 output slices. No cross-core communication.

On-chip layout: activations are feature-major (x^T [D, tokens]) so every
linear layer chains on the PE without transposes. Softmax is computed as
scores^T [keys, queries]; the denominator comes free by appending a
ones-column to V (row 65 of the AV accumulation). Matmuls run in bf16
with f32 PSUM accumulation; residuals/LN in f32.
"""

import numpy as np
import ml_dtypes

import concourse.bacc as bacc
import concourse.mybir as mybir
from concourse.tile import TileContext

DT = mybir.dt
BF = DT.bfloat16
F32 = DT.float32

B = 2
S = 2048          # keys per sequence
QTOK = 512        # query tokens per core
D = 1024
H = 16
DK = 64
FF = 4096
DC = D // 128     # 8  feature chunks
FC = FF // 128    # 32 ffn chunks
KC = S // 128     # 16 key chunks
EPS = 1e-5
N_CORES = 8
INV_SQRT_DK = 0.125

Alu = mybir.AluOpType
Act = mybir.ActivationFunctionType


def _build_nc():
    nc = bacc.Bacc()

    xT = nc.dram_tensor("xT", [D, S], BF, kind="ExternalInput")
    xqT = nc.dram_tensor("xqT", [D, QTOK], F32, kind="ExternalInput")
    wq = nc.dram_tensor("wq", [D, D], BF, kind="ExternalInput")
    wk = nc.dram_tensor("wk", [D, D], BF, kind="ExternalInput")
    wv = nc.dram_tensor("wv", [D, D], BF, kind="ExternalInput")
    wo = nc.dram_tensor("wo", [D, D], BF, kind="ExternalInput")
    w1 = nc.dram_tensor("w1", [D, FF], BF, kind="ExternalInput")
    w2 = nc.dram_tensor("w2", [FF, D], BF, kind="ExternalInput")
    bo = nc.dram_tensor("bo", [D], F32, kind="ExternalInput")
    b1 = nc.dram_tensor("b1", [FF], F32, kind="ExternalInput")
    b2 = nc.dram_tensor("b2", [D], F32, kind="ExternalInput")
    g1 = nc.dram_tensor("g1", [D], F32, kind="ExternalInput")
    be1 = nc.dram_tensor("be1", [D], F32, kind="ExternalInput")
    g2 = nc.dram_tensor("g2", [D], F32, kind="ExternalInput")
    be2 = nc.dram_tensor("be2", [D], F32, kind="ExternalInput")
    yT = nc.dram_tensor("yT", [D, QTOK], F32, kind="ExternalOutput")

    xT_d = xT.rearrange("(c p) t -> p c t", p=128)
    xqT_d = xqT.rearrange("(c p) t -> p c t", p=128)
    wq_d = wq.rearrange("(c p) m -> p c m", p=128)
    wk_d = wk.rearrange("(c p) m -> p c m", p=128)
    wv_d = wv.rearrange("(c p) m -> p c m", p=128)
    wo_d = wo.rearrange("(c p) m -> p c m", p=128)
    w1_d = w1.rearrange("(c p) m -> p c m", p=128)
    w2_d = w2.rearrange("(c p) m -> p c m", p=128)
    yT_d = yT.rearrange("(c p) t -> p c t", p=128)

    with TileContext(nc) as tc:
        with (
            tc.tile_pool(name="const", bufs=1) as const,
            tc.tile_pool(name="arena", bufs=1) as arena,
            tc.tile_pool(name="epool", bufs=2) as epool,
            tc.tile_pool(name="wpool", bufs=2) as wpool,
            tc.tile_pool(name="w8pool", bufs=2) as w8pool,
            tc.tile_pool(name="bpool", bufs=2) as bpool,
            tc.tile_pool(name="tpool", bufs=2) as tpool,
            tc.tile_pool(name="spool", bufs=1) as spool,
            tc.tile_pool(name="ps", bufs=3, space="PSUM") as ps,
            tc.tile_pool(name="avps", bufs=2, space="PSUM") as avps,
            tc.tile_pool(name="stps", bufs=1, space="PSUM") as stps,
        ):
            # ---- constants / params ----
            xqT_sb = const.tile([128, DC, QTOK], F32)
            nc.sync.dma_start(out=xqT_sb, in_=xqT_d)
            bo_sb = const.tile([128, DC], F32)
            nc.sync.dma_start(out=bo_sb, in_=bo.rearrange("(c p) -> p c", p=128))
            b1_sb = const.tile([128, FC], F32)
            nc.sync.dma_start(out=b1_sb, in_=b1.rearrange("(c p) -> p c", p=128))
            b2_sb = const.tile([128, DC], F32)
            nc.sync.dma_start(out=b2_sb, in_=b2.rearrange("(c p) -> p c", p=128))
            g1_sb = const.tile([128, DC], F32)
            nc.sync.dma_start(out=g1_sb, in_=g1.rearrange("(c p) -> p c", p=128))
            be1_sb = const.tile([128, DC], F32)
            nc.sync.dma_start(out=be1_sb, in_=be1.rearrange("(c p) -> p c", p=128))
            g2_sb = const.tile([128, DC], F32)
            nc.sync.dma_start(out=g2_sb, in_=g2.rearrange("(c p) -> p c", p=128))
            be2_sb = const.tile([128, DC], F32)
            nc.sync.dma_start(out=be2_sb, in_=be2.rearrange("(c p) -> p c", p=128))
            ones_sb = const.tile([128, 1], BF)
            nc.vector.memset(ones_sb, 1.0)
            eps_sb = const.tile([1, 1], F32)
            nc.vector.memset(eps_sb, EPS)

            # ---- arena tiles (tag-based reuse; bufs=1 slots) ----
            xT_sb = arena.tile([128, DC, S], BF, tag="A")      # 32K/part
            nc.sync.dma_start(out=xT_sb, in_=xT_d)
            kT_sb = arena.tile([128, DC, S], BF, tag="B")      # 32K
            qT_sb = arena.tile([128, DC, QTOK], BF, tag="C")   # 8K
            v_sb = arena.tile([128, KC, H * 65], BF, tag="V")  # 32.5K
            v4 = v_sb.rearrange("p k (h c) -> p k h c", c=65)
            ctxb_sb = arena.tile([128, DC, QTOK], BF, tag="G")  # 8K

            # ---- P1: K^T and Q^T (feature-major) ----
            for wdram, dst, ntok in ((wk_d, kT_sb, S), (wq_d, qT_sb, QTOK)):
                for f in range(DC):
                    wt = wpool.tile([128, DC, 128], BF, tag="w")
                    nc.sync.dma_start(out=wt, in_=wdram[:, :, f * 128:(f + 1) * 128])
                    for t in range(ntok // 512):
                        mm = ps.tile([128, 512], F32, tag="mm")
                        for d in range(DC):
                            nc.tensor.matmul(
                                mm,
                                lhsT=wt[:, d, :],
                                rhs=xT_sb[:, d, t * 512:(t + 1) * 512],
                                start=(d == 0),
                                stop=(d == DC - 1),
                            )
                        nc.vector.tensor_copy(dst[:, f, t * 512:(t + 1) * 512], mm)

            # ---- P1b: V natural [tokens, feats] with ones column ----
            for half in range(2):
                wt5 = w8pool.tile([128, DC, 512], BF, tag="w8")
                nc.sync.dma_start(out=wt5, in_=wv_d[:, :, half * 512:(half + 1) * 512])
                for t in range(KC):
                    mm = ps.tile([128, 512], F32, tag="mm")
                    for d in range(DC):
                        nc.tensor.matmul(
                            mm,
                            lhsT=xT_sb[:, d, t * 128:(t + 1) * 128],
                            rhs=wt5[:, d, :],
                            start=(d == 0),
                            stop=(d == DC - 1),
                        )
                    nc.vector.tensor_copy(
                        v4[:, t, half * 8:(half + 1) * 8, 0:64],
                        mm.rearrange("p (h c) -> p h c", c=64),
                    )
            nc.vector.memset(v4[:, :, :, 64:65], 1.0)

            # ---- P2: attention per head ----
            for h in range(H):
                hc, p0 = h // 2, (h % 2) * 64
                av = avps.tile([128, QTOK], F32, tag="av")
                for kc in range(KC):
                    mm = ps.tile([128, QTOK], F32, tag="mm")
                    nc.tensor.matmul(
                        mm,
                        lhsT=kT_sb[p0:p0 + 64, hc, kc * 128:(kc + 1) * 128],
                        rhs=qT_sb[p0:p0 + 64, hc, :],
                        start=True,
                        stop=True,
                    )
                    eT = epool.tile([128, QTOK], BF, tag="e")
                    nc.scalar.activation(eT, mm, Act.Exp, scale=INV_SQRT_DK)
                    nc.tensor.matmul(
                        av[0:65, :],
                        lhsT=v4[:, kc, h, :],
                        rhs=eT,
                        start=(kc == 0),
                        stop=(kc == KC - 1),
                    )
                hr = bpool.tile([1, QTOK], F32, tag="b")
                nc.vector.reciprocal(hr, av[64:65, :])
                hb = bpool.tile([128, QTOK], F32, tag="b")
                nc.gpsimd.partition_broadcast(hb[0:64, :], hr, channels=64)
                nc.vector.tensor_mul(
                    ctxb_sb[p0:p0 + 64, hc, :], av[0:64, :], hb[0:64, :]
                )

            # ---- P3: Wo projection + residual ----
            r1_sb = arena.tile([128, DC, QTOK], F32, tag="V")  # reuse V slot
            for j in range(DC):
                wt = wpool.tile([128, DC, 128], BF, tag="w")
                nc.sync.dma_start(out=wt, in_=wo_d[:, :, j * 128:(j + 1) * 128])
                mm = ps.tile([128, QTOK], F32, tag="mm")
                for d in range(DC):
                    nc.tensor.matmul(
                        mm,
                        lhsT=wt[:, d, :],
                        rhs=ctxb_sb[:, d, :],
                        start=(d == 0),
                        stop=(d == DC - 1),
                    )
                # r1 = (mm + bo) + xq
                nc.vector.scalar_tensor_tensor(
                    r1_sb[:, j, :],
                    mm,
                    bo_sb[:, j:j + 1],
                    xqT_sb[:, j, :],
                    Alu.add,
                    Alu.add,
                )

            # ---- P4: LayerNorm 1 (feature dim = partitions, via ones-matmul) ----
            def layer_norm(src_f32, gam, bet, out_f32, out_bf16):
                # tag reuse: "G" held ctxb (dead after Wo), "C" held qT /
                # x1b (lnsq's writes WAR-wait on prior readers; safe since
                # stats must finish before the affine stage anyway)
                srcb = arena.tile([128, DC, QTOK], BF, tag="G")
                srcsq = arena.tile([128, DC, QTOK], BF, tag="C")
                nc.vector.tensor_copy(srcb, src_f32)
                nc.vector.tensor_mul(srcsq, srcb, srcb)
                sum_ps = stps.tile([1, QTOK], F32, tag="sum")
                sq_ps = stps.tile([1, QTOK], F32, tag="sq")
                for d in range(DC):
                    nc.tensor.matmul(
                        sum_ps, lhsT=ones_sb, rhs=srcb[:, d, :],
                        start=(d == 0), stop=(d == DC - 1),
                    )
                for d in range(DC):
                    nc.tensor.matmul(
                        sq_ps, lhsT=ones_sb, rhs=srcsq[:, d, :],
                        start=(d == 0), stop=(d == DC - 1),
                    )
                st = spool.tile([1, 3, QTOK], F32, tag="st")
                mu, ex2, mu2 = st[0:1, 0, :], st[0:1, 1, :], st[0:1, 2, :]
                var, sd, rstd = st[0:1, 2, :], st[0:1, 1, :], st[0:1, 2, :]
                nc.scalar.activation(mu, sum_ps, Act.Copy, scale=1.0 / D)
                nc.scalar.activation(ex2, sq_ps, Act.Copy, scale=1.0 / D)
                nc.vector.tensor_mul(mu2, mu, mu)
                nc.vector.tensor_sub(var, ex2, mu2)
                nc.scalar.activation(sd, var, Act.Sqrt, bias=eps_sb, scale=1.0)
                nc.vector.reciprocal(rstd, sd)
                mub = bpool.tile([128, QTOK], F32, tag="b")
                nc.gpsimd.partition_broadcast(mub, mu, channels=128)
                rsb = bpool.tile([128, QTOK], F32, tag="b")
                nc.gpsimd.partition_broadcast(rsb, rstd, channels=128)
                for d in range(DC):
                    t1 = tpool.tile([128, QTOK], F32, tag="t1")
                    nc.vector.tensor_sub(t1, src_f32[:, d, :], mub)
                    t2 = t1
                    nc.vector.tensor_mul(t2, t1, rsb)
                    if out_f32 is not None:
                        nc.scalar.activation(
                            out_f32[:, d, :], t2, Act.Identity,
                            bias=bet[:, d:d + 1], scale=gam[:, d:d + 1],
                        )
                    if out_bf16 is not None:
                        nc.vector.tensor_scalar(
                            out_bf16[:, d, :], t2,
                            gam[:, d:d + 1], bet[:, d:d + 1],
                            Alu.mult, Alu.add,
                        )

            x1_sb = arena.tile([128, DC, QTOK], F32, tag="A")  # reuse xT slot
            x1b_sb = arena.tile([128, DC, QTOK], BF, tag="C")  # reuse qT slot
            layer_norm(r1_sb, g1_sb, be1_sb, x1_sb, x1b_sb)

            # ---- P5: FFN ----
            h_sb = arena.tile([128, FC, QTOK], BF, tag="B")  # reuse kT slot
            for f in range(FC):
                wt = wpool.tile([128, DC, 128], BF, tag="w")
                nc.sync.dma_start(out=wt, in_=w1_d[:, :, f * 128:(f + 1) * 128])
                mm = ps.tile([128, QTOK], F32, tag="mm")
                for d in range(DC):
                    nc.tensor.matmul(
                        mm,
                        lhsT=wt[:, d, :],
                        rhs=x1b_sb[:, d, :],
                        start=(d == 0),
                        stop=(d == DC - 1),
                    )
                nc.scalar.activation(
                    h_sb[:, f, :], mm, Act.Relu, bias=b1_sb[:, f:f + 1], scale=1.0
                )

            r2_sb = arena.tile([128, DC, QTOK], F32, tag="F")  # reuse ctx slot
            for j in range(DC):
                w2t = w8pool.tile([128, FC, 128], BF, tag="w8")
                nc.sync.dma_start(out=w2t, in_=w2_d[:, :, j * 128:(j + 1) * 128])
                mm = ps.tile([128, QTOK], F32, tag="mm")
                for fc in range(FC):
                    nc.tensor.matmul(
                        mm,
                        lhsT=w2t[:, fc, :],
                        rhs=h_sb[:, fc, :],
                        start=(fc == 0),
                        stop=(fc == FC - 1),
                    )
                nc.vector.scalar_tensor_tensor(
                    r2_sb[:, j, :],
                    mm,
                    b2_sb[:, j:j + 1],
                    x1_sb[:, j, :],
                    Alu.add,
                    Alu.add,
                )

            # ---- P6: LayerNorm 2 -> output ----
            yT_sb = arena.tile([128, DC, QTOK], F32, tag="B2")
            layer_norm(r2_sb, g2_sb, be2_sb, yT_sb, None)
            nc.sync.dma_start(out=yT_d, in_=yT_sb)

    nc.compile()
    return nc


_CACHE = {}


def _get_runner():
    """Build + compile once; return a cached callable mapping
    list-of-8 in_maps -> list-of-8 out_maps, mirroring
    bass2jax.run_bass_via_pjrt's multi-core path."""
    if "runner" in _CACHE:
        return _CACHE["runner"]

    import jax
    import jax.numpy as jnp  # noqa: F401
    from jax.sharding import Mesh, PartitionSpec
    from jax.experimental.shard_map import shard_map
    from concourse import bass2jax
    from concourse import mybir as _mybir

    bass2jax.install_neuronx_cc_hook()
    nc = _build_nc()

    partition_name = (
        nc.partition_id_tensor.name if nc.partition_id_tensor else None
    )
    in_names, out_names, out_avals, zero_outs = [], [], [], []
    for alloc in nc.m.functions[0].allocations:
        if not isinstance(alloc, _mybir.MemoryLocationSet):
            continue
        name = alloc.memorylocations[0].name
        if alloc.kind == "ExternalInput":
            if name != partition_name:
                in_names.append(name)
        elif alloc.kind == "ExternalOutput":
            shape = tuple(alloc.tensor_shape)
            dtype = _mybir.dt.np(alloc.dtype)
            out_avals.append(jax.core.ShapedArray(shape, dtype))
            out_names.append(name)
            zero_outs.append(np.zeros(shape, dtype))
    n_params = len(in_names)
    all_in_names = list(in_names) + list(out_names)
    if partition_name is not None:
        all_in_names.append(partition_name)

    donate = tuple(range(n_params, n_params + len(out_names)))

    def _body(*args):
        operands = list(args)
        if partition_name is not None:
            operands.append(bass2jax.partition_id_tensor())
        outs = bass2jax._bass_exec_p.bind(
            *operands,
            out_avals=tuple(out_avals),
            in_names=tuple(all_in_names),
            out_names=tuple(out_names),
            lowering_input_output_aliases=(),
            sim_require_finite=True,
            sim_require_nnan=True,
            nc=nc,
        )
        return tuple(outs)

    devices = jax.devices()[:N_CORES]
    mesh = Mesh(np.asarray(devices), ("core",))
    in_specs = (PartitionSpec("core"),) * (n_params + len(out_names))
    out_specs = (PartitionSpec("core"),) * len(out_names)
    sharded = jax.jit(
        shard_map(
            _body, mesh=mesh, in_specs=in_specs, out_specs=out_specs,
            check_rep=False,
        ),
        donate_argnums=donate,
        keep_unused=True,
    )

    def run(in_maps):
        per_core = [[np.asarray(m[n]) for n in in_names] for m in in_maps]
        concat_in = [
            np.concatenate([per_core[c][i] for c in range(N_CORES)], axis=0)
            for i in range(n_params)
        ]
        concat_zeros = [
            np.zeros((N_CORES * z.shape[0], *z.shape[1:]), z.dtype)
            for z in zero_outs
        ]
        out_arrs = sharded(*concat_in, *concat_zeros)
        return [
            {
                name: np.asarray(out_arrs[i]).reshape(
                    N_CORES, *out_avals[i].shape
                )[c]
                for i, name in enumerate(out_names)
            }
            for c in range(N_CORES)
        ]

    _CACHE["runner"] = (run, sharded, in_names, out_names, out_avals, n_params, zero_outs)
    return _CACHE["runner"]


def _prep_in_maps(x, Wq, Wk, Wv, Wo, bo, W1, b1, W2, b2, g1, be1, g2, be2):
    bf = ml_dtypes.bfloat16
    shared = {
        "wq": np.ascontiguousarray(Wq.astype(bf)),
        "wk": np.ascontiguousarray(Wk.astype(bf)),
        "wv": np.ascontiguousarray(Wv.astype(bf)),
        "wo": np.ascontiguousarray(Wo.astype(bf)),
        "w1": np.ascontiguousarray(W1.astype(bf)),
        "w2": np.ascontiguousarray(W2.astype(bf)),
        "bo": np.ascontiguousarray(bo.astype(np.float32)),
        "b1": np.ascontiguousarray(b1.astype(np.float32)),
        "b2": np.ascontiguousarray(b2.astype(np.float32)),
        "g1": np.ascontiguousarray(g1.astype(np.float32)),
        "be1": np.ascontiguousarray(be1.astype(np.float32)),
        "g2": np.ascontiguousarray(g2.astype(np.float32)),
        "be2": np.ascontiguousarray(be2.astype(np.float32)),
    }
    in_maps = []
    for c in range(N_CORES):
        b, r = c // 4, c % 4
        xb = np.roll(np.asarray(x[b], np.float32), -QTOK * r, axis=0)
        m = dict(shared)
        m["xT"] = np.ascontiguousarray(xb.T.astype(bf))
        m["xqT"] = np.ascontiguousarray(xb[:QTOK].T.astype(np.float32))
        in_maps.append(m)
    return in_maps


def kernel(**inputs):
    x = np.asarray(inputs["x"], np.float32)
    in_maps = _prep_in_maps(
        x,
        inputs["Wq"], inputs["Wk"], inputs["Wv"], inputs["Wo"], inputs["bo"],
        inputs["W1"], inputs["b1"], inputs["W2"], inputs["b2"],
        inputs["g1"], inputs["be1"], inputs["g2"], inputs["be2"],
    )
    run = _get_runner()[0]
    outs = run(in_maps)
    out = np.empty((B, S, D), np.float32)
    for c in range(N_CORES):
        b, r = c // 4, c % 4
        out[b, QTOK * r:QTOK * (r + 1)] = outs[c]["yT"].T
    return out



# revision 8
# speedup vs baseline: 150.6273x; 150.6273x over previous
"""Trainium2 Bass kernel for a post-LN transformer encoder block.

Problem: x[2,2048,1024], 16 heads, FFN 4096, mask all-False.

Sharding (zero-collective sequence parallel):
  8 cores = 2 batch elements x 4 query slices of 512 tokens.
  Each core computes K/V for the full 2048-token sequence of its batch
  element, attention + FFN for its 512 query tokens only.

Precision plan (validated numerically, rms_rel ~ 6e-3 vs 2e-2 gate):
  - x and Wq/Wk/Wv/Wo are fp8e4 (weights pre-scaled x32 on host).
    QKV + Wo projections and the AV matmul run in fp8 DoubleRow mode
    (K=256 contraction per instruction, 0.5 cycles/row).
  - Scores stay bf16 (DK=64 contraction cannot DoubleRow); the 32x32
    scale of kT/qT folds into the exp activation scale; exp outputs
    fp8 e/32 via a -ln(32) bias fold.  V is stored as fp8 32*V so the
    AV product is exactly Sum(e*v); the softmax denominator comes from
    a DoubleRow ones-matmul, and ctx = av/denom is 32*ctx in fp8 which
    is exactly the scale Wo-DR wants.  All descales fold into existing
    instructions (1/1024 in the Wo residual STT, 1/8192 in exp scale).
  - FFN stays bf16 (fp8 FFN fails the accuracy gate).
  - Softmax/LN reciprocals use reciprocal_approx_fast (18 bits, ~5x
    cheaper than the exact single-lane reciprocal).
  - All DRAM operands are host-preshuffled so every DMA is a contiguous
    per-partition run (the naive strided layout costs 100k+ tiny DMA
    packets).
"""

import math

import numpy as np
import ml_dtypes

import concourse.bacc as bacc
import concourse.mybir as mybir
from concourse.tile import TileContext

DT = mybir.dt
BF = DT.bfloat16
F32 = DT.float32
F8 = DT.float8e4

B = 2
S = 2048          # keys per sequence
QTOK = 512        # query tokens per core
D = 1024
H = 16
DK = 64
FF = 4096
DC = D // 128     # 8  feature chunks
FC = FF // 128    # 32 ffn chunks
KC = S // 128     # 16 key chunks
KP = KC // 2      # 8  key-chunk pairs (DoubleRow)
EPS = 1e-5
N_CORES = 8
WS = 32.0                      # host weight prescale for fp8
EXP_SCALE = 0.125 / (WS * WS)  # 1/sqrt(DK) / (32*32)
EXP_BIAS = -math.log(WS)       # exp outputs e/32 (fp8-safe range)

Alu = mybir.AluOpType
Act = mybir.ActivationFunctionType
DR = mybir.MatmulPerfMode.DoubleRow


def _build_nc():
    nc = bacc.Bacc()

    x8d = nc.dram_tensor("x8", [128, 4 * 2 * S], F8, kind="ExternalInput")
    xqbd = nc.dram_tensor("xqb", [128, DC * QTOK], F32, kind="ExternalInput")
    wq8d = nc.dram_tensor("wq8", [128, 4 * 2 * D], F8, kind="ExternalInput")
    wk8d = nc.dram_tensor("wk8", [128, 4 * 2 * D], F8, kind="ExternalInput")
    wv8d = nc.dram_tensor("wv8", [128, 4 * 2 * D], F8, kind="ExternalInput")
    wo8d = nc.dram_tensor("wo8", [128, 4 * 2 * D], F8, kind="ExternalInput")
    w1rd = nc.dram_tensor("w1r", [128, DC * DC * 512], BF, kind="ExternalInput")
    w2rd = nc.dram_tensor("w2r", [128, DC * FC * 128], BF, kind="ExternalInput")
    b1rd = nc.dram_tensor("b1r", [128, FC], F32, kind="ExternalInput")
    g1rd = nc.dram_tensor("g1r", [128, DC], F32, kind="ExternalInput")
    be1rd = nc.dram_tensor("be1r", [128, DC], F32, kind="ExternalInput")
    bxrd = nc.dram_tensor("bxr", [128, DC], F32, kind="ExternalInput")  # be1+b2
    g2rd = nc.dram_tensor("g2r", [128, DC], F32, kind="ExternalInput")
    be2rd = nc.dram_tensor("be2r", [128, DC], F32, kind="ExternalInput")
    yTd = nc.dram_tensor("yT", [D, QTOK], F32, kind="ExternalOutput")
    yT_v = yTd.rearrange("(c p) t -> p c t", p=128)

    with TileContext(nc) as tc:
        with (
            tc.tile_pool(name="const", bufs=1) as const,
            tc.tile_pool(name="arena", bufs=1) as arena,
            tc.tile_pool(name="wqkvo", bufs=1) as wqkvo,
            tc.tile_pool(name="wff", bufs=2) as wff,
            tc.tile_pool(name="epool", bufs=3) as epool,
            tc.tile_pool(name="bpool", bufs=2) as bpool,
            tc.tile_pool(name="spool", bufs=1) as spool,
            tc.tile_pool(name="ps", bufs=2, space="PSUM") as ps,
        ):
            # ---- constants / params ----
            xqb_sb = const.tile([128, DC, QTOK], F32)
            nc.sync.dma_start(out=xqb_sb, in_=xqbd.rearrange("p (c t) -> p c t", t=QTOK))
            b1_sb = const.tile([128, FC], F32)
            nc.sync.dma_start(out=b1_sb, in_=b1rd.rearrange("p c -> p c"))
            g1_sb = const.tile([128, DC], F32)
            nc.sync.dma_start(out=g1_sb, in_=g1rd.rearrange("p c -> p c"))
            be1_sb = const.tile([128, DC], F32)
            nc.sync.dma_start(out=be1_sb, in_=be1rd.rearrange("p c -> p c"))
            bx_sb = const.tile([128, DC], F32)
            nc.sync.dma_start(out=bx_sb, in_=bxrd.rearrange("p c -> p c"))
            g2_sb = const.tile([128, DC], F32)
            nc.sync.dma_start(out=g2_sb, in_=g2rd.rearrange("p c -> p c"))
            be2_sb = const.tile([128, DC], F32)
            nc.sync.dma_start(out=be2_sb, in_=be2rd.rearrange("p c -> p c"))
            ones_sb = const.tile([128, 1], BF)
            nc.vector.memset(ones_sb, 1.0)
            ones8_sb = const.tile([128, 2, 16], F8)
            nc.vector.memset(ones8_sb, 1.0)
            eps_sb = const.tile([1, 1], F32)
            nc.vector.memset(eps_sb, EPS)
            expb_sb = const.tile([128, 1], F32)
            nc.vector.memset(expb_sb, EXP_BIAS)

            # ---- fp8 weights (fully resident) ----
            wq8 = wqkvo.tile([128, 4, 2, D], F8, tag="wq")
            nc.sync.dma_start(out=wq8, in_=wq8d.rearrange("p (a i f) -> p a i f", i=2, f=D))
            wk8 = wqkvo.tile([128, 4, 2, D], F8, tag="wk")
            nc.sync.dma_start(out=wk8, in_=wk8d.rearrange("p (a i f) -> p a i f", i=2, f=D))
            wv8 = wqkvo.tile([128, 4, 2, D], F8, tag="wv")
            nc.sync.dma_start(out=wv8, in_=wv8d.rearrange("p (a i f) -> p a i f", i=2, f=D))
            wo8 = wqkvo.tile([128, 4, 2, D], F8, tag="wo")
            nc.sync.dma_start(out=wo8, in_=wo8d.rearrange("p (a i f) -> p a i f", i=2, f=D))

            # ---- arena tiles ----
            x8_sb = arena.tile([128, 4, 2, S], F8, tag="X")       # 16K/part
            nc.sync.dma_start(out=x8_sb, in_=x8d.rearrange("p (a i t) -> p a i t", i=2, t=S))
            kT_sb = arena.tile([128, DC, S], BF, tag="K")         # 32K
            qT_sb = arena.tile([128, DC, QTOK], BF, tag="Q")      # 8K
            v4 = arena.tile([128, KP, 2, H, DK], F8, tag="V")     # 16K
            ctx8 = arena.tile([128, 4, 2, QTOK], F8, tag="C")     # 4K (tag max 8K)

            w1r_v = w1rd.rearrange("p (fb c f) -> p fb c f", c=DC, f=512)
            w2r_v = w2rd.rearrange("p (j c f) -> p j c f", c=FC, f=128)

            # ---- P1: K^T and Q^T (feature-major, bf16, values x32) ----
            # DoubleRow outputs must sit at partition base 0 (<=64 rows), so
            # each 128-feature chunk is two 64-row chains into the two banks
            # of one PSUM tile, merged by partition-shifting copies.
            for wt, dst, ntok in ((wk8, kT_sb, S), (wq8, qT_sb, QTOK)):
                for f in range(DC):
                    for t in range(ntok // 512):
                        mm = ps.tile([128, 2, 512], F32, tag="big")
                        for mh in range(2):
                            for pr in range(4):
                                nc.tensor.matmul(
                                    mm[0:64, mh, :],
                                    lhsT=wt[:, pr, :, f * 128 + mh * 64:f * 128 + (mh + 1) * 64],
                                    rhs=x8_sb[:, pr, :, t * 512:(t + 1) * 512],
                                    start=(pr == 0),
                                    stop=(pr == 3),
                                    perf_mode=DR,
                                )
                        nc.vector.tensor_copy(
                            dst[0:64, f, t * 512:(t + 1) * 512], mm[0:64, 0, :])
                        nc.vector.tensor_copy(
                            dst[64:128, f, t * 512:(t + 1) * 512], mm[0:64, 1, :])

            # ---- P1b: V natural [tokens, feats] fp8 (values x32) ----
            for tch in range(KC):
                for fh in range(2):
                    mm = ps.tile([128, 2, 512], F32, tag="big")
                    for mh in range(2):
                        for pr in range(4):
                            nc.tensor.matmul(
                                mm[0:64, mh, :],
                                lhsT=x8_sb[:, pr, :, tch * 128 + mh * 64:tch * 128 + (mh + 1) * 64],
                                rhs=wv8[:, pr, :, fh * 512:(fh + 1) * 512],
                                start=(pr == 0),
                                stop=(pr == 3),
                                perf_mode=DR,
                            )
                    for mh in range(2):
                        nc.scalar.activation(
                            v4[mh * 64:(mh + 1) * 64, tch // 2, tch % 2,
                               fh * 8:(fh + 1) * 8, :].rearrange("p h c -> p (h c)"),
                            mm[0:64, mh, :], Act.Copy, scale=1.0,
                        )

            # ---- P2: attention per head ----
            for h in range(H):
                hc, p0 = h // 2, (h % 2) * 64
                av = ps.tile([128, 512], F32, tag="av")
                den = ps.tile([1, 512], F32, tag="den")
                for kp in range(KP):
                    sc = ps.tile([128, 2, 512], F32, tag="big")
                    for i in range(2):
                        nc.tensor.matmul(
                            sc[:, i, :],
                            lhsT=kT_sb[p0:p0 + 64, hc,
                                       (2 * kp + i) * 128:(2 * kp + i + 1) * 128],
                            rhs=qT_sb[p0:p0 + 64, hc, :],
                            start=True,
                            stop=True,
                        )
                    eT = epool.tile([128, 2, 512], F8, tag="e")
                    nc.scalar.activation(eT, sc, Act.Exp, scale=EXP_SCALE, bias=expb_sb)
                    nc.tensor.matmul(
                        av[0:64, :],
                        lhsT=v4[:, kp, :, h, :],
                        rhs=eT,
                        start=(kp == 0),
                        stop=(kp == KP - 1),
                        perf_mode=DR,
                    )
                    nc.tensor.matmul(
                        den,
                        lhsT=ones8_sb[:, :, 0:1],
                        rhs=eT,
                        start=(kp == 0),
                        stop=(kp == KP - 1),
                        perf_mode=DR,
                    )
                hr = bpool.tile([1, QTOK], F32, tag="r")
                nc.vector.reciprocal_approx_fast(hr, den)
                hb = bpool.tile([128, QTOK], F32, tag="b")
                nc.gpsimd.partition_broadcast(hb[0:64, :], hr, channels=64)
                nc.vector.tensor_mul(
                    ctx8[(h % 2) * 64:(h % 2) * 64 + 64, h // 4, (h // 2) % 2, :],
                    av[0:64, :], hb[0:64, :],
                )

            # ---- P3: Wo projection (DR) + residual ----
            r1_sb = arena.tile([128, DC, QTOK], F32, tag="X")  # x8 dead
            for j in range(DC):
                mm = ps.tile([128, 2, 512], F32, tag="big")
                for mh in range(2):
                    for pr in range(4):
                        nc.tensor.matmul(
                            mm[0:64, mh, :],
                            lhsT=wo8[:, pr, :, j * 128 + mh * 64:j * 128 + (mh + 1) * 64],
                            rhs=ctx8[:, pr, :, :],
                            start=(pr == 0),
                            stop=(pr == 3),
                            perf_mode=DR,
                        )
                for mh in range(2):
                    nc.vector.scalar_tensor_tensor(
                        r1_sb[mh * 64:(mh + 1) * 64, j, :], mm[0:64, mh, :],
                        1.0 / (WS * WS), xqb_sb[mh * 64:(mh + 1) * 64, j, :],
                        Alu.mult, Alu.add,
                    )

            # ---- LayerNorm (stats via ones-matmul over partitions) ----
            def layer_norm(src_f32, gam, bet_f32, bet_bf, out_f32, out_bf16):
                srcb = arena.tile([128, DC, QTOK], BF, tag="C")
                srcsq = arena.tile([128, DC, QTOK], BF, tag="D")
                nc.vector.tensor_copy(srcb, src_f32)
                nc.vector.tensor_mul(srcsq, srcb, srcb)
                sum_ps = ps.tile([1, QTOK], F32, tag="big")
                sq_ps = ps.tile([1, QTOK], F32, tag="big")
                for d in range(DC):
                    nc.tensor.matmul(
                        sum_ps, lhsT=ones_sb, rhs=srcb[:, d, :],
                        start=(d == 0), stop=(d == DC - 1),
                    )
                for d in range(DC):
                    nc.tensor.matmul(
                        sq_ps, lhsT=ones_sb, rhs=srcsq[:, d, :],
                        start=(d == 0), stop=(d == DC - 1),
                    )
                st = spool.tile([1, 3, QTOK], F32, tag="st")
                mu, ex2, mu2 = st[0:1, 0, :], st[0:1, 1, :], st[0:1, 2, :]
                var, sd, rstd = st[0:1, 2, :], st[0:1, 1, :], st[0:1, 2, :]
                nc.scalar.activation(mu, sum_ps, Act.Copy, scale=1.0 / D)
                nc.scalar.activation(ex2, sq_ps, Act.Copy, scale=1.0 / D)
                nc.vector.tensor_mul(mu2, mu, mu)
                nc.vector.tensor_sub(var, ex2, mu2)
                nc.scalar.activation(sd, var, Act.Sqrt, bias=eps_sb, scale=1.0)
                nc.vector.reciprocal_approx_fast(rstd, sd)
                mub = bpool.tile([128, QTOK], F32, tag="b")
                nc.gpsimd.partition_broadcast(mub, mu, channels=128)
                rsb = bpool.tile([128, QTOK], F32, tag="b")
                nc.gpsimd.partition_broadcast(rsb, rstd, channels=128)
                for d in range(DC):
                    t1 = bpool.tile([128, QTOK], F32, tag="t1")
                    nc.vector.tensor_sub(t1, src_f32[:, d, :], mub)
                    nc.vector.tensor_mul(t1, t1, rsb)
                    if out_f32 is not None:
                        nc.vector.tensor_scalar(
                            out_f32[:, d, :], t1,
                            gam[:, d:d + 1], bet_f32[:, d:d + 1],
                            Alu.mult, Alu.add,
                        )
                    if out_bf16 is not None:
                        nc.scalar.activation(
                            out_bf16[:, d, :], t1, Act.Identity,
                            bias=bet_bf[:, d:d + 1], scale=gam[:, d:d + 1],
                        )

            x1_sb = arena.tile([128, DC, QTOK], F32, tag="V")   # v4 dead
            x1b_sb = arena.tile([128, DC, QTOK], BF, tag="Q")   # qT dead
            # x1 f32 carries be1+b2 (for the FFN2 residual); x1b carries be1.
            layer_norm(r1_sb, g1_sb, bx_sb, be1_sb, x1_sb, x1b_sb)

            # ---- P5: FFN1 (bf16) ----
            h_sb = arena.tile([128, FC, QTOK], BF, tag="K")  # kT dead
            for fb in range(DC):
                w1t = wff.tile([128, DC, 512], BF, tag="w1")
                nc.sync.dma_start(out=w1t, in_=w1r_v[:, fb])
                for fp2 in range(2):
                    mm = ps.tile([128, 2, 512], F32, tag="big")
                    for half in range(2):
                        fc = fb * 4 + fp2 * 2 + half
                        for d in range(DC):
                            nc.tensor.matmul(
                                mm[:, half, :],
                                lhsT=w1t[:, d, (fp2 * 2 + half) * 128:(fp2 * 2 + half + 1) * 128],
                                rhs=x1b_sb[:, d, :],
                                start=(d == 0),
                                stop=(d == DC - 1),
                            )
                    for half in range(2):
                        fc = fb * 4 + fp2 * 2 + half
                        nc.scalar.activation(
                            h_sb[:, fc, :], mm[:, half, :], Act.Relu,
                            bias=b1_sb[:, fc:fc + 1], scale=1.0,
                        )

            # ---- P6: FFN2 (bf16) + residual (b2 folded into x1) ----
            r2_sb = arena.tile([128, DC, QTOK], F32, tag="X")  # r1 dead
            for jp in range(DC // 2):
                mm = ps.tile([128, 2, 512], F32, tag="big")
                for half in range(2):
                    j = jp * 2 + half
                    w2t = wff.tile([128, FC, 128], BF, tag="w2")
                    nc.sync.dma_start(out=w2t, in_=w2r_v[:, j])
                    for fc in range(FC):
                        nc.tensor.matmul(
                            mm[:, half, :],
                            lhsT=w2t[:, fc, :],
                            rhs=h_sb[:, fc, :],
                            start=(fc == 0),
                            stop=(fc == FC - 1),
                        )
                nc.vector.tensor_add(
                    r2_sb[:, jp * 2:jp * 2 + 2, :], mm, x1_sb[:, jp * 2:jp * 2 + 2, :])

            # ---- P7: LayerNorm 2 -> output ----
            yT_sb = arena.tile([128, DC, QTOK], F32, tag="V")  # x1 dead
            layer_norm(r2_sb, g2_sb, be2_sb, None, yT_sb, None)
            nc.sync.dma_start(out=yT_v, in_=yT_sb)

    nc.compile()
    return nc


_CACHE = {}


def _get_runner():
    """Build + compile once; return a cached callable mapping
    list-of-8 in_maps -> list-of-8 out_maps."""
    if "runner" in _CACHE:
        return _CACHE["runner"]

    import jax
    from jax.sharding import Mesh, PartitionSpec
    from jax.experimental.shard_map import shard_map
    from concourse import bass2jax
    from concourse import mybir as _mybir

    bass2jax.install_neuronx_cc_hook()
    nc = _build_nc()

    partition_name = (
        nc.partition_id_tensor.name if nc.partition_id_tensor else None
    )
    in_names, out_names, out_avals, zero_outs = [], [], [], []
    for alloc in nc.m.functions[0].allocations:
        if not isinstance(alloc, _mybir.MemoryLocationSet):
            continue
        name = alloc.memorylocations[0].name
        if alloc.kind == "ExternalInput":
            if name != partition_name:
                in_names.append(name)
        elif alloc.kind == "ExternalOutput":
            shape = tuple(alloc.tensor_shape)
            dtype = _mybir.dt.np(alloc.dtype)
            out_avals.append(jax.core.ShapedArray(shape, dtype))
            out_names.append(name)
            zero_outs.append(np.zeros(shape, dtype))
    n_params = len(in_names)
    all_in_names = list(in_names) + list(out_names)
    if partition_name is not None:
        all_in_names.append(partition_name)

    donate = tuple(range(n_params, n_params + len(out_names)))

    def _body(*args):
        operands = list(args)
        if partition_name is not None:
            operands.append(bass2jax.partition_id_tensor())
        outs = bass2jax._bass_exec_p.bind(
            *operands,
            out_avals=tuple(out_avals),
            in_names=tuple(all_in_names),
            out_names=tuple(out_names),
            lowering_input_output_aliases=(),
            sim_require_finite=True,
            sim_require_nnan=True,
            nc=nc,
        )
        return tuple(outs)

    devices = jax.devices()[:N_CORES]
    mesh = Mesh(np.asarray(devices), ("core",))
    in_specs = (PartitionSpec("core"),) * (n_params + len(out_names))
    out_specs = (PartitionSpec("core"),) * len(out_names)
    sharded = jax.jit(
        shard_map(
            _body, mesh=mesh, in_specs=in_specs, out_specs=out_specs,
            check_rep=False,
        ),
        donate_argnums=donate,
        keep_unused=True,
    )

    def run(in_maps):
        per_core = [[np.asarray(m[n]) for n in in_names] for m in in_maps]
        concat_in = [
            np.concatenate([per_core[c][i] for c in range(N_CORES)], axis=0)
            for i in range(n_params)
        ]
        concat_zeros = [
            np.zeros((N_CORES * z.shape[0], *z.shape[1:]), z.dtype)
            for z in zero_outs
        ]
        out_arrs = sharded(*concat_in, *concat_zeros)
        return [
            {
                name: np.asarray(out_arrs[i]).reshape(
                    N_CORES, *out_avals[i].shape
                )[c]
                for i, name in enumerate(out_names)
            }
            for c in range(N_CORES)
        ]

    _CACHE["runner"] = (run, sharded, in_names, out_names, out_avals, n_params, zero_outs)
    return _CACHE["runner"]


def _prep_in_maps(x, Wq, Wk, Wv, Wo, bo, W1, b1, W2, b2, g1, be1, g2, be2):
    bf = ml_dtypes.bfloat16
    f8 = ml_dtypes.float8_e4m3

    def w8_shuffle(W):
        # [128, 4, 2, 1024] fp8: [p, pair, i, f] = 32*W[pair*256+i*128+p, f]
        a = (np.asarray(W, np.float32) * WS).reshape(4, 2, 128, D)
        return np.ascontiguousarray(
            a.transpose(2, 0, 1, 3).reshape(128, 4 * 2 * D).astype(f8))

    w1r = np.ascontiguousarray(
        np.asarray(W1, np.float32).reshape(DC, 128, DC, 512)
        .transpose(1, 2, 0, 3).reshape(128, DC * DC * 512).astype(bf))
    w2r = np.ascontiguousarray(
        np.asarray(W2, np.float32).reshape(FC, 128, DC, 128)
        .transpose(1, 2, 0, 3).reshape(128, DC * FC * 128).astype(bf))

    def col128(v):
        return np.ascontiguousarray(
            np.asarray(v, np.float32).reshape(-1, 128).T)

    shared = {
        "wq8": w8_shuffle(Wq),
        "wk8": w8_shuffle(Wk),
        "wv8": w8_shuffle(Wv),
        "wo8": w8_shuffle(Wo),
        "w1r": w1r,
        "w2r": w2r,
        "b1r": col128(b1),
        "g1r": col128(g1),
        "be1r": col128(be1),
        "bxr": col128(np.asarray(be1, np.float32) + np.asarray(b2, np.float32)),
        "g2r": col128(g2),
        "be2r": col128(be2),
    }
    bo32 = np.asarray(bo, np.float32)
    in_maps = []
    for c in range(N_CORES):
        b, r = c // 4, c % 4
        xb = np.roll(np.asarray(x[b], np.float32), -QTOK * r, axis=0)
        m = dict(shared)
        m["x8"] = np.ascontiguousarray(
            xb.T.reshape(4, 2, 128, S).transpose(2, 0, 1, 3)
            .reshape(128, 4 * 2 * S).astype(f8))
        m["xqb"] = np.ascontiguousarray(
            (xb[:QTOK] + bo32).T.reshape(DC, 128, QTOK)
            .transpose(1, 0, 2).reshape(128, DC * QTOK))
        in_maps.append(m)
    return in_maps


def kernel(**inputs):
    x = np.asarray(inputs["x"], np.float32)
    in_maps = _prep_in_maps(
        x,
        inputs["Wq"], inputs["Wk"], inputs["Wv"], inputs["Wo"], inputs["bo"],
        inputs["W1"], inputs["b1"], inputs["W2"], inputs["b2"],
        inputs["g1"], inputs["be1"], inputs["g2"], inputs["be2"],
    )
    run = _get_runner()[0]
    outs = run(in_maps)
    out = np.empty((B, S, D), np.float32)
    for c in range(N_CORES):
        b, r = c // 4, c % 4
        out[b, QTOK * r:QTOK * (r + 1)] = (
            outs[c]["yT"].reshape(D, QTOK).T)
    return out
